# revision 1
# baseline (speedup 1.0000x reference)
"""Averaged Hausdorff loss kernel for Trainium2 (8 NeuronCores, SPMD).

Computes mean(min_j d(x_i, y_j)) + mean(min_i d(x_i, y_j)) for
set1 [8192, 256], set2 [8192, 256] using the Gram trick:
    d2[i,j] = ||x_i||^2 + ||y_j||^2 - 2 <x_i, y_j>

Sharding: set1 rows split across 8 cores (1024 rows each); every core
holds all of set2 and computes its [1024 x 8192] distance tile. Row-mins
are complete per core; column-mins are partial and min-reduced on host
(8 x 8192 values — trivial) before the final sqrt/means.

Per-core layout (transposed gram, gT[j, i], q = ||.||^2/2):
    PE     : psum = gT - q1[i]   (q1 folded in via a K=1 augmentation
             matmul: lhsT = ones[1,128], rhs = -q1 row)
    DVE    : cacc[:, jt] = max_i psum          -> column mins
    DVE    : racc = max(psum - q2[j], racc) over jt -> row mins (finished
             by a partition-max via PE transposes)
    colmin_d2[j] = 2*(q2[j] - cacc[j]);  rowmin_d2[i] = -2*max_p racc[p,i]
sqrt is monotone so all mins are taken on d2 and sqrt'd on host.
"""

import os
import sys

import numpy as np

for _p in ("/opt/trn_rl_repo", os.path.expanduser("~/.axon_site/_ro/trn_rl_repo")):
    if os.path.isdir(_p) and _p not in sys.path:
        sys.path.insert(0, _p)

import concourse.bass as bass
import concourse.mybir as mybir
from concourse import bacc
from concourse.masks import make_identity
from concourse.tile import TileContext

N1 = 8192  # set1 rows
N2 = 8192  # set2 rows
D = 256    # feature dim
NCORES = 8
SHARD = N1 // NCORES   # 1024 set1 rows per core
P = 128
RT = SHARD // P        # 8 row(i)-tiles per core
JT = N2 // P           # 64 col(j)-tiles
KT = D // P            # 2 contraction chunks
F32 = mybir.dt.float32
NEG = -1.0e30
HALF_SQRT = 0.70710677  # sqrt(0.5) in f32; Square(x*s) = x^2/2
H16 = mybir.dt.float16  # accumulator / aug-operand dtype (precision-pinned)

A = mybir.AluOpType
AX = mybir.AxisListType
AF = mybir.ActivationFunctionType


def build_kernel(mm_dt=mybir.dt.float16):
    """`mm_dt` is the matmul operand dtype: float16 (full 1-cycle/row PE
    rate, ~1e-4 output error), float32r (~2 cycles/row measured, ~1e-6),
    or float32 (exact, 4 cycles/row)."""
    nc = bacc.Bacc()
    s1 = nc.declare_dram_parameter("s1", [SHARD, D], F32, isOutput=False)
    s2 = nc.declare_dram_parameter("s2", [N2, D], F32, isOutput=False)
    rowmin = nc.declare_dram_parameter("rowmin", [P, RT], F32, isOutput=True)
    colmin = nc.declare_dram_parameter("colmin", [P, JT], F32, isOutput=True)

    with TileContext(nc) as tc:
        with (
            tc.tile_pool(name="persist", bufs=1) as persist,
            tc.tile_pool(name="stage", bufs=4) as stage,
            tc.tile_pool(name="junk", bufs=3) as junkp,
            tc.tile_pool(name="vpool", bufs=3) as vpool,
        ):
            # persistent SBUF tensors
            s2T = [persist.tile([P, N2], mm_dt, name=f"s2T{k}", tag=f"s2T{k}") for k in range(KT)]
            s1T = [persist.tile([P, SHARD], mm_dt, name=f"s1T{k}", tag=f"s1T{k}") for k in range(KT)]
            q2 = persist.tile([P, JT], F32)       # ||y||^2/2, j-natural layout
            nq2 = persist.tile([P, JT], F32)      # -q2 (ACT bias for v)
            negq1row = persist.tile([1, SHARD], H16)  # -||x||^2/2 along free
            ones_row = persist.tile([1, P], H16)      # aug-matmul lhsT
            racc = [persist.tile([P, SHARD], H16, name=f"racc{k}", tag=f"racc{k}") for k in range(2)]
            cacc = persist.tile([P, JT], F32)     # max_i (gT - q1) per j
            ident_f = persist.tile([P, P], F32)
            ident_h = persist.tile([P, P], H16)
            consts_f = persist.tile([P, 3], F32)  # cols: 0.5, -0.5, 1.0
            neghalf = persist.tile([P, 1], H16)

            make_identity(nc, ident_f)
            make_identity(nc, ident_h)
            nc.gpsimd.memset(consts_f[:, 0:1], 0.5)
            nc.gpsimd.memset(consts_f[:, 1:2], -0.5)
            nc.gpsimd.memset(consts_f[:, 2:3], 1.0)
            nc.gpsimd.memset(racc[0], -60000.0)
            # fp32r constants must be produced by a rounding instruction
            nc.vector.tensor_copy(neghalf, consts_f[:, 1:2])
            nc.vector.tensor_copy(ones_row, consts_f[0:1, 2:3].to_broadcast((1, P)))

            # ---- Stage A: transpose inputs, squared norms, aug operands ----
            with tc.tile_pool(name="psA", bufs=4, space="PSUM") as psA:
                GRP = 4  # 128-row blocks transposed per PSUM tile / DVE copy

                def load_transpose(src, nt_tiles, dstT, qdst):
                    """src [rows, D] -> dstT[k] [P, rows] (k = d chunk);
                    qdst[:, t] = ||row||^2/2 in row-natural layout (optional)."""
                    for g in range(nt_tiles // GRP):
                        nats = []
                        for b in range(GRP):
                            t = g * GRP + b
                            nat = stage.tile([P, D], F32, tag="nat")
                            nc.sync.dma_start(nat, src[t * P:(t + 1) * P, :])
                            nats.append((t, nat))
                        if qdst is not None:
                            # squared norms on ACT: accum = sum(x^2/2)
                            for t, nat in nats:
                                aj = junkp.tile([P, D], F32, tag="actjunk")
                                nc.scalar.activation(
                                    aj, nat, AF.Square, scale=HALF_SQRT,
                                    accum_out=qdst[:, t:t + 1],
                                )
                        for k in range(KT):
                            pt = psA.tile([P, GRP * P], F32, tag="ptrans")
                            for b, (t, nat) in enumerate(nats):
                                nc.tensor.transpose(
                                    pt[:, b * P:(b + 1) * P],
                                    nat[:, k * P:(k + 1) * P],
                                    ident_f,
                                )
                            nc.scalar.copy(
                                dstT[k][:, g * GRP * P:(g + 1) * GRP * P], pt
                            )

                load_transpose(s2, JT, s2T, q2)
                load_transpose(s1, RT, s1T, None)
                nc.vector.tensor_scalar_mul(nq2, q2, -1.0)

                # negq1row[0, i] = -||x_i||^2/2 via all-(-0.5) matmul on
                # squared s1T chunks: sum_d -0.5 * s1T[d,i]^2.
                s1sq = [stage.tile([P, SHARD], H16, name=f"s1sq{k}", tag=f"s1sq{k}") for k in range(KT)]
                for k in range(KT):
                    nc.scalar.square(s1sq[k], s1T[k])
                for blk in range(SHARD // 512):
                    pr = psA.tile([1, 512], F32, tag="prep")
                    for k in range(KT):
                        nc.tensor.matmul(
                            pr, lhsT=neghalf, rhs=s1sq[k][:, blk * 512:(blk + 1) * 512],
                            start=(k == 0), stop=(k == KT - 1),
                        )
                    nc.vector.tensor_copy(negq1row[0:1, blk * 512:(blk + 1) * 512], pr)

            # ---- Stage B: gram tiles + fused min-reductions ----
            with tc.tile_pool(name="psB", bufs=3, space="PSUM") as psB:
                for jt in range(JT):
                    pg = psB.tile([P, SHARD], F32, tag="gram")
                    for half in range(SHARD // 512):
                        sl = slice(half * 512, (half + 1) * 512)
                        for k in range(KT):
                            nc.tensor.matmul(
                                pg[:, sl],
                                lhsT=s2T[k][:, jt * P:(jt + 1) * P],
                                rhs=s1T[k][:, sl],
                                start=(k == 0), stop=False,
                            )
                        # fold -q1[i] into the gram: += ones.T @ (-q1 row)
                        nc.tensor.matmul(
                            pg[:, sl], lhsT=ones_row, rhs=negq1row[0:1, sl],
                            start=False, stop=True,
                        )
                    # col pass (DVE): cacc[:, jt] = max_i (gT - q1)
                    nc.vector.tensor_reduce(
                        cacc[:, jt:jt + 1], pg, axis=AX.X, op=A.max
                    )
                    # row pass: ACT applies -q2[j] while moving PSUM->SBUF
                    # (fp16), then DVE max-accumulates at 2-byte 2x rate
                    v = vpool.tile([P, SHARD], H16, tag="v")
                    nc.scalar.activation(v, pg, AF.Identity, bias=nq2[:, jt:jt + 1])
                    nc.vector.tensor_tensor(
                        racc[(jt + 1) % 2], v, racc[jt % 2], A.max
                    )

            # ---- Stage C: finalize ----
            with tc.tile_pool(name="psC", bufs=2, space="PSUM") as psC:
                rfin = racc[JT % 2]
                m1 = persist.tile([P, RT], F32)
                for b in range(RT):
                    ptc = psC.tile([P, P], mm_dt, tag="ptc")
                    nc.tensor.transpose(ptc, rfin[:, b * P:(b + 1) * P], ident_h)
                    nc.vector.tensor_reduce(
                        m1[:, b:b + 1], ptc, axis=AX.X, op=A.max
                    )
                # rowmin_d2 = -2*m1; colmin_d2 = 2*(q2 - cacc)
                rfix = persist.tile([P, RT], F32)
                cfix = persist.tile([P, JT], F32)
                nc.vector.tensor_scalar_mul(rfix, m1, -2.0)
                nc.vector.tensor_tensor(cfix, q2, cacc, A.subtract)
                nc.vector.tensor_scalar_mul(cfix, cfix, 2.0)
                nc.sync.dma_start(rowmin[:, :], rfix)
                nc.sync.dma_start(colmin[:, :], cfix)

    nc.compile()
    return nc


_CACHE: dict = {}


def _built(mm_dt_name: str):
    if mm_dt_name not in _CACHE:
        _CACHE[mm_dt_name] = build_kernel(getattr(mybir.dt, mm_dt_name))
    return _CACHE[mm_dt_name]


def run_on_cores(set1: np.ndarray, set2: np.ndarray, mm_dt_name="float16", **kw):
    """Run the SPMD kernel; returns (rowmin_d2 [8192], colmin_d2 [8192], bass_results)."""
    from concourse.bass_utils import run_bass_kernel_spmd

    nc = _built(mm_dt_name)
    in_maps = [
        {
            "s1": np.ascontiguousarray(set1[c * SHARD:(c + 1) * SHARD]),
            "s2": np.ascontiguousarray(set2),
        }
        for c in range(NCORES)
    ]
    res = run_bass_kernel_spmd(nc, in_maps, core_ids=list(range(NCORES)), **kw)
    row_parts, col_parts = [], []
    for c in range(NCORES):
        # [P, T] with element (p, t) = row/col index t*P + p
        row_parts.append(res.results[c]["rowmin"].T.reshape(-1))
        col_parts.append(res.results[c]["colmin"].T.reshape(-1))
    rowmin_d2 = np.concatenate(row_parts)            # [8192], by global row
    colmin_d2 = np.min(np.stack(col_parts), axis=0)  # [8192], min over cores
    return rowmin_d2, colmin_d2, res


def kernel(set1, set2) -> np.ndarray:
    set1 = np.asarray(set1, dtype=np.float32)
    set2 = np.asarray(set2, dtype=np.float32)
    rowmin_d2, colmin_d2, _ = run_on_cores(set1, set2)
    t1 = np.sqrt(np.maximum(rowmin_d2, 0.0), dtype=np.float32).mean(dtype=np.float32)
    t2 = np.sqrt(np.maximum(colmin_d2, 0.0), dtype=np.float32).mean(dtype=np.float32)
    return np.array(np.float32(t1) + np.float32(t2), dtype=np.float32)



# revision 12
# speedup vs baseline: 1.6118x; 1.6118x over previous
"""Averaged Hausdorff loss kernel for Trainium2 (8 NeuronCores, SPMD).

Computes mean(min_j d(x_i, y_j)) + mean(min_i d(x_i, y_j)) for
set1 [8192, 256], set2 [8192, 256] using the Gram trick:
    d2[i,j] = ||x_i||^2 + ||y_j||^2 - 2 <x_i, y_j>

Sharding: set1 rows split across 8 cores (1024 rows each); every core
holds all of set2 and computes its [1024 x 8192] distance tile. Row-mins
are complete per core; column-mins are partial and min-reduced on host
(8 x 8192 values) before the final sqrt/means.

Per-core pipeline (q = ||.||^2/2, all mins on d2/2, sqrt'd on host):
    PE   : pg[j,i] = <x_i,y_j> - q1[i], via ONE fp8e4 DoubleRow matmul
           (K=256 in a single instruction) per 512-column half plus a
           K=1 fp16 aug matmul (ones x -q1row); augs of 4 consecutive
           halves sit at base partitions 0/32/64/96 so they pack into
           disjoint PE row-groups and run concurrently.
    ACT  : v = pg + (-q2[j]) bias, PSUM f32 -> SBUF fp16.
    DVE  : cacc[:,jt] = max_i v (tensor_tensor_reduce, fused max+reduce)
           racc = max(v, racc) elementwise (fp16 2x mode).
    GPSIMD: q2 column norms from the f32 natural tiles (off critical path).
    colmin_d2 = -2*cacc;  rowmin_d2 = -2*max_p racc (PE transposes).
"""

import os
import sys

import numpy as np

for _p in ("/opt/trn_rl_repo", os.path.expanduser("~/.axon_site/_ro/trn_rl_repo")):
    if os.path.isdir(_p) and _p not in sys.path:
        sys.path.insert(0, _p)

import concourse.bass as bass
import concourse.mybir as mybir
from concourse import bacc
from concourse.masks import make_identity
from concourse.tile import TileContext

N1 = 8192  # set1 rows
N2 = 8192  # set2 rows
D = 256    # feature dim
NCORES = 8
SHARD = N1 // NCORES   # 1024 set1 rows per core
P = 128
RT = SHARD // P        # 8 row(i)-tiles per core
JT = N2 // P           # 64 col(j)-tiles
KT = D // P            # 2 contraction chunks
F32 = mybir.dt.float32
F16 = mybir.dt.float16
FP8 = mybir.dt.float8e4
NEG = -60000.0
HALF_SQRT = 0.70710677  # sqrt(0.5); Square(x*s) = x^2/2

A = mybir.AluOpType
AX = mybir.AxisListType
AF = mybir.ActivationFunctionType
PM = mybir.MatmulPerfMode

GRP = 4  # natural 128-row tiles converted/transposed per batch


def build_kernel(use_ttr=True, use_doublerow=True):
    nc = bacc.Bacc()
    s1 = nc.declare_dram_parameter("s1", [SHARD, D], F32, isOutput=False)
    s2 = nc.declare_dram_parameter("s2", [N2, D], F32, isOutput=False)
    rowmin = nc.declare_dram_parameter("rowmin", [P, RT], F32, isOutput=True)
    colmin = nc.declare_dram_parameter("colmin", [P, JT], F32, isOutput=True)

    with TileContext(nc) as tc:
        with (
            tc.tile_pool(name="persist", bufs=1) as persist,
            tc.tile_pool(name="nat32", bufs=24) as nat32p,   # f32 natural tiles
            tc.tile_pool(name="nat16", bufs=3) as nat16p,    # fp16 natural (pre-transpose)
            tc.tile_pool(name="vpool", bufs=3) as vpool,
            tc.tile_pool(name="junk", bufs=2) as junkp,
        ):
            # ---- persistent SBUF tensors ----
            # transposed fp8 operands, chunk-major: [P, k, cols]
            s2T8 = persist.tile([P, KT, N2], FP8)
            s1T8 = persist.tile([P, KT, SHARD], FP8)
            nq2 = persist.tile([P, JT], F32)        # -||y||^2/2 per j (ACT bias)
            q2pos = persist.tile([P, JT], F32)      # +q2 staging for ACT-accum tiles
            negq1rep = persist.tile([P, SHARD], F16)  # -q1 row at partitions 0/32/64/96
            onesrep = persist.tile([P, P], F16)       # ones rows at partitions 0/32/64/96
            racc = [persist.tile([P, SHARD], F16, name=f"racc{k}", tag=f"racc{k}") for k in range(2)]
            cacc = persist.tile([P, JT], F32)
            ident_h = persist.tile([P, P], F16)
            neghalf = persist.tile([P, 1], F16)
            consts_f = persist.tile([P, 2], F32)  # cols: -0.5, 1.0
            s1sq = persist.tile([P, KT, SHARD], F16)
            m1 = persist.tile([P, RT], F32)
            rfix = persist.tile([P, RT], F32)
            cfix = persist.tile([P, JT], F32)

            make_identity(nc, ident_h)
            nc.gpsimd.memset(consts_f[:, 0:1], -0.5)
            nc.gpsimd.memset(consts_f[:, 1:2], 1.0)
            nc.gpsimd.memset(racc[0], NEG)
            nc.gpsimd.memset(onesrep, 1.0)
            nc.vector.tensor_copy(neghalf, consts_f[:, 0:1])

            def load_convert_transpose(src, t0, ntiles, dstT, psA, q_ops):
                """DMA f32 tiles [t0, t0+ntiles), convert to fp16, PE-transpose
                per k-chunk into one fp16 PSUM tile, evacuate as fp8 into
                dstT[:, k, t0*P : (t0+ntiles)*P]. q_ops(tile_idx, nat32_tile)
                runs per natural tile (for norms)."""
                nat32 = nat32p.tile([P, ntiles * D], F32, tag="nat32")
                for b in range(ntiles):
                    t = t0 + b
                    nc.sync.dma_start(
                        nat32[:, b * D:(b + 1) * D], src[t * P:(t + 1) * P, :]
                    )
                nat16 = nat16p.tile([P, ntiles * D], F16, tag="nat16")
                nc.vector.tensor_copy(nat16, nat32)
                for b in range(ntiles):
                    q_ops(t0 + b, nat32[:, b * D:(b + 1) * D])
                # transpose: psum layout [k, b*P : ...]
                pt = psA.tile([P, KT, ntiles * P], F16, tag="ptrans")
                for k in range(KT):
                    for b in range(ntiles):
                        nc.tensor.transpose(
                            pt[:, k, b * P:(b + 1) * P],
                            nat16[:, b * D + k * P: b * D + (k + 1) * P],
                            ident_h,
                        )
                # evacuate to fp8, 3D AP: out [P, KT, ntiles*P]
                dst = dstT[:, :, t0 * P:(t0 + ntiles) * P]
                if (t0 // ntiles) % 2 == 0:
                    nc.scalar.copy(dst, pt)
                else:
                    nc.vector.tensor_copy(dst, pt)

            def q2_ops(t, nat):
                junk = junkp.tile([P, D], F32, tag="q2junk")
                if t % 2 == 0:
                    # ACT: junk = (x*sqrt(.5))^2, accum = +q2 (negated later)
                    nc.scalar.activation(
                        junk, nat, AF.Square, scale=HALF_SQRT,
                        accum_out=q2pos[:, t:t + 1],
                    )
                else:
                    # DVE: junk = (x * -0.5) * x, accum = -q2 directly
                    nc.vector.scalar_tensor_tensor(
                        out=junk, in0=nat, scalar=-0.5, in1=nat,
                        op0=A.mult, op1=A.mult,
                        accum_out=nq2[:, t:t + 1],
                    )

            def no_q(t, nat):
                pass

            with tc.tile_pool(name="psA", bufs=2, space="PSUM") as psA:
                # ---- s1 first (stage B needs all of it) ----
                for g in range(RT // GRP):
                    load_convert_transpose(s1, g * GRP, GRP, s1T8, psA, no_q)

                # negq1row: -q1 via neghalf^T @ square(s1T8); replicated to
                # partitions 0/32/64/96 of negq1rep for packed aug matmuls.
                nc.scalar.square(s1sq, s1T8)
                with tc.tile_pool(name="psP", bufs=2, space="PSUM") as psP:
                    for blk in range(SHARD // 512):
                        pr = psP.tile([1, 512], F32, tag="prep")
                        for k in range(KT):
                            nc.tensor.matmul(
                                pr,
                                lhsT=neghalf,
                                rhs=s1sq[:, k, blk * 512:(blk + 1) * 512],
                                start=(k == 0), stop=(k == KT - 1),
                            )
                        for rep in range(4):
                            nc.vector.tensor_copy(
                                negq1rep[32 * rep:32 * rep + 1, blk * 512:(blk + 1) * 512],
                                pr,
                            )

                # ---- s2 groups + interleaved stage B ----
                with tc.tile_pool(name="psB", bufs=3, space="PSUM") as psB:
                    HALF = 512
                    NH = SHARD // HALF  # 2 halves per jt

                    def stage_b(jt_pair):
                        """Emit PE+ACT+DVE work for jt_pair = [jt, jt+1]."""
                        pgs = []
                        for jt in jt_pair:
                            pg = psB.tile([P, SHARD], F32, tag="gram")
                            pgs.append((jt, pg))
                            for h in range(NH):
                                sl = slice(h * HALF, (h + 1) * HALF)
                                if use_doublerow:
                                    nc.tensor.matmul(
                                        pg[:, sl],
                                        lhsT=s2T8[:, :, jt * P:(jt + 1) * P],
                                        rhs=s1T8[:, :, sl],
                                        start=True, stop=False,
                                        perf_mode=PM.DoubleRow,
                                    )
                                else:
                                    for k in range(KT):
                                        nc.tensor.matmul(
                                            pg[:, sl],
                                            lhsT=s2T8[:, k, jt * P:(jt + 1) * P],
                                            rhs=s1T8[:, k, sl],
                                            start=(k == 0), stop=False,
                                        )
                        # packed augs: 4 (jt, half) slots at partitions 0/32/64/96
                        slot = 0
                        for jt, pg in pgs:
                            for h in range(NH):
                                sl = slice(h * HALF, (h + 1) * HALF)
                                bp = 32 * slot
                                nc.tensor.matmul(
                                    pg[:, sl],
                                    lhsT=onesrep[bp:bp + 1, :],
                                    rhs=negq1rep[bp:bp + 1, sl],
                                    start=False, stop=True,
                                    tile_position=(bp, 0),
                                )
                                slot += 1
                        # ACT evac + bias, then DVE reductions
                        for jt, pg in pgs:
                            v = vpool.tile([P, SHARD], F16, tag="v")
                            nc.scalar.activation(
                                v, pg, AF.Identity, bias=nq2[:, jt:jt + 1]
                            )
                            if use_ttr:
                                w = junkp.tile([P, SHARD], F16, tag="w")
                                nc.vector.tensor_tensor_reduce(
                                    out=w, in0=v, in1=v, scale=1.0,
                                    scalar=NEG, op0=A.max, op1=A.max,
                                    accum_out=cacc[:, jt:jt + 1],
                                )
                            else:
                                nc.vector.tensor_reduce(
                                    cacc[:, jt:jt + 1], v, axis=AX.X, op=A.max
                                )
                            nc.vector.tensor_tensor(
                                racc[(jt + 1) % 2], v, racc[jt % 2], A.max
                            )

                    for g in range(JT // GRP):
                        load_convert_transpose(s2, g * GRP, GRP, s2T8, psA, q2_ops)
                        # negate the ACT-accumulated (even) q2 columns: [P, 2] strided
                        nc.vector.tensor_scalar_mul(
                            nq2[:, g * GRP:(g + 1) * GRP:2],
                            q2pos[:, g * GRP:(g + 1) * GRP:2],
                            -1.0,
                        )
                        for half_pair in range(GRP // 2):
                            jt = g * GRP + half_pair * 2
                            stage_b([jt, jt + 1])

            # ---- finalize ----
            with tc.tile_pool(name="psC", bufs=2, space="PSUM") as psC:
                rfin = racc[JT % 2]
                for b in range(RT):
                    ptc = psC.tile([P, P], F16, tag="ptc")
                    nc.tensor.transpose(ptc, rfin[:, b * P:(b + 1) * P], ident_h)
                    nc.vector.tensor_reduce(
                        m1[:, b:b + 1], ptc, axis=AX.X, op=A.max
                    )
                # rowmin_d2 = -2*m1; colmin_d2 = -2*cacc
                nc.vector.tensor_scalar_mul(rfix, m1, -2.0)
                nc.vector.tensor_scalar_mul(cfix, cacc, -2.0)
                nc.sync.dma_start(rowmin[:, :], rfix)
                nc.sync.dma_start(colmin[:, :], cfix)

    nc.compile()
    return nc


_CACHE: dict = {}


def _built(key=("ttr", "dr")):
    if key not in _CACHE:
        _CACHE[key] = build_kernel(
            use_ttr="ttr" in key, use_doublerow="dr" in key
        )
    return _CACHE[key]


def run_on_cores(set1: np.ndarray, set2: np.ndarray, variant=("ttr", "dr"), **kw):
    """Run the SPMD kernel; returns (rowmin_d2 [8192], colmin_d2 [8192], results)."""
    from concourse.bass_utils import run_bass_kernel_spmd

    nc = _built(variant)
    in_maps = [
        {
            "s1": np.ascontiguousarray(set1[c * SHARD:(c + 1) * SHARD]),
            "s2": np.ascontiguousarray(set2),
        }
        for c in range(NCORES)
    ]
    res = run_bass_kernel_spmd(nc, in_maps, core_ids=list(range(NCORES)), **kw)
    row_parts, col_parts = [], []
    for c in range(NCORES):
        # [P, T] with element (p, t) = row/col index t*P + p
        row_parts.append(res.results[c]["rowmin"].T.reshape(-1))
        col_parts.append(res.results[c]["colmin"].T.reshape(-1))
    rowmin_d2 = np.concatenate(row_parts)            # [8192], by global row
    colmin_d2 = np.min(np.stack(col_parts), axis=0)  # [8192], min over cores
    return rowmin_d2, colmin_d2, res


def kernel(set1, set2) -> np.ndarray:
    set1 = np.asarray(set1, dtype=np.float32)
    set2 = np.asarray(set2, dtype=np.float32)
    rowmin_d2, colmin_d2, _ = run_on_cores(set1, set2)
    t1 = np.sqrt(np.maximum(rowmin_d2, 0.0), dtype=np.float32).mean(dtype=np.float32)
    t2 = np.sqrt(np.maximum(colmin_d2, 0.0), dtype=np.float32).mean(dtype=np.float32)
    return np.array(np.float32(t1) + np.float32(t2), dtype=np.float32)


# revision 19
# speedup vs baseline: 1.7120x; 1.0622x over previous
"""Averaged Hausdorff loss kernel for Trainium2 (8 NeuronCores, SPMD).

Computes mean(min_j d(x_i, y_j)) + mean(min_i d(x_i, y_j)) for
set1 [8192, 256], set2 [8192, 256] using the Gram trick:
    d2[i,j] = ||x_i||^2 + ||y_j||^2 - 2 <x_i, y_j>

Sharding: set1 rows split across 8 cores (1024 rows each); every core
holds all of set2 and computes its [1024 x 8192] distance tile. Row-mins
are complete per core; column-mins are partial and min-reduced on host
(8 x 8192 values) before the final sqrt/means.

Per-core pipeline (q = ||.||^2/2, all mins on d2/2, sqrt'd on host):
    PE   : pg[j,i] = <x_i,y_j> - q1[i], via ONE fp8e4 DoubleRow matmul
           (K=256 in a single instruction) per 512-column half plus a
           K=1 fp16 aug matmul (ones x -q1row); augs of 4 consecutive
           halves sit at base partitions 0/32/64/96 so they pack into
           disjoint PE row-groups and run concurrently.
    ACT  : v = pg + (-q2[j]) bias, PSUM f32 -> SBUF fp16.
    DVE  : cacc[:,jt] = max_i v (tensor_tensor_reduce, fused max+reduce)
           racc = max(v, racc) elementwise (fp16 2x mode).
    GPSIMD: q2 column norms from the f32 natural tiles (off critical path).
    colmin_d2 = -2*cacc;  rowmin_d2 = -2*max_p racc (PE transposes).
"""

import os
import sys

import numpy as np

for _p in ("/opt/trn_rl_repo", os.path.expanduser("~/.axon_site/_ro/trn_rl_repo")):
    if os.path.isdir(_p) and _p not in sys.path:
        sys.path.insert(0, _p)

import concourse.bass as bass
import concourse.mybir as mybir
from concourse import bacc
from concourse.masks import make_identity
from concourse.tile import TileContext

N1 = 8192  # set1 rows
N2 = 8192  # set2 rows
D = 256    # feature dim
NCORES = 8
SHARD = N1 // NCORES   # 1024 set1 rows per core
P = 128
RT = SHARD // P        # 8 row(i)-tiles per core
JT = N2 // P           # 64 col(j)-tiles
KT = D // P            # 2 contraction chunks
F32 = mybir.dt.float32
F16 = mybir.dt.float16
FP8 = mybir.dt.float8e4
NEG = -60000.0
HALF_SQRT = 0.70710677  # sqrt(0.5); Square(x*s) = x^2/2

A = mybir.AluOpType
AX = mybir.AxisListType
AF = mybir.ActivationFunctionType
PM = mybir.MatmulPerfMode

GRP = 4  # natural 128-row tiles converted/transposed per batch


def build_kernel(use_ttr=True, use_doublerow=True, use_gpcvt=False):
    nc = bacc.Bacc()
    s1 = nc.declare_dram_parameter("s1", [SHARD, D], F32, isOutput=False)
    s2 = nc.declare_dram_parameter("s2", [N2, D], F32, isOutput=False)
    rowmin = nc.declare_dram_parameter("rowmin", [P, RT], F32, isOutput=True)
    colmin = nc.declare_dram_parameter("colmin", [P, JT], F32, isOutput=True)

    with TileContext(nc) as tc:
        with (
            tc.tile_pool(name="persist", bufs=1) as persist,
            tc.tile_pool(name="nat32", bufs=24) as nat32p,   # f32 natural tiles
            tc.tile_pool(name="nat16", bufs=3) as nat16p,    # fp16 natural (pre-transpose)
            tc.tile_pool(name="vpool", bufs=3) as vpool,
            tc.tile_pool(name="junk", bufs=2) as junkp,
        ):
            # ---- persistent SBUF tensors ----
            # transposed fp8 operands, chunk-major: [P, k, cols]
            s2T8 = persist.tile([P, KT, N2], FP8)
            s1T8 = persist.tile([P, KT, SHARD], FP8)
            nq2 = persist.tile([P, JT], F32)        # -||y||^2/2 per j (ACT bias)
            q2pos = persist.tile([P, JT], F32)      # +q2 staging for ACT-accum tiles
            negq1rep = persist.tile([P, SHARD], F16)  # -q1 row at partitions 0/32/64/96
            onesrep = persist.tile([P, P], F16)       # ones rows at partitions 0/32/64/96
            racc = [persist.tile([P, SHARD], F16, name=f"racc{k}", tag=f"racc{k}") for k in range(2)]
            negtile = persist.tile([P, SHARD], F16)  # -inf-ish constant (ttr in1)
            cacc = persist.tile([P, JT], F32)
            ident_h = persist.tile([P, P], F16)
            neghalf = persist.tile([P, 1], F16)
            consts_f = persist.tile([P, 2], F32)  # cols: -0.5, 1.0
            s1sq = persist.tile([P, KT, SHARD], F16)
            m1 = persist.tile([P, RT], F32)
            rfix = persist.tile([P, RT], F32)
            cfix = persist.tile([P, JT], F32)

            make_identity(nc, ident_h)
            nc.gpsimd.memset(consts_f[:, 0:1], -0.5)
            nc.gpsimd.memset(consts_f[:, 1:2], 1.0)
            nc.gpsimd.memset(racc[0], NEG)
            nc.gpsimd.memset(negtile, NEG)
            nc.gpsimd.memset(onesrep, 1.0)
            nc.vector.tensor_copy(neghalf, consts_f[:, 0:1])

            def load_convert_transpose(src, t0, ntiles, dstT, psA, q_ops):
                """DMA f32 tiles [t0, t0+ntiles), convert to fp16, PE-transpose
                per k-chunk into one fp16 PSUM tile, evacuate as fp8 into
                dstT[:, k, t0*P : (t0+ntiles)*P]. q_ops(tile_idx, nat32_tile)
                runs per natural tile (for norms)."""
                nat32 = nat32p.tile([P, ntiles * D], F32, tag="nat32")
                for b in range(ntiles):
                    t = t0 + b
                    nc.sync.dma_start(
                        nat32[:, b * D:(b + 1) * D], src[t * P:(t + 1) * P, :]
                    )
                nat16 = nat16p.tile([P, ntiles * D], F16, tag="nat16")
                if use_gpcvt:
                    nc.gpsimd.tensor_copy(nat16, nat32)
                else:
                    nc.vector.tensor_copy(nat16, nat32)
                for b in range(ntiles):
                    q_ops(t0 + b, nat32[:, b * D:(b + 1) * D])
                # transpose: psum layout [k, b*P : ...]
                pt = psA.tile([P, KT, ntiles * P], F16, tag="ptrans")
                for k in range(KT):
                    for b in range(ntiles):
                        nc.tensor.transpose(
                            pt[:, k, b * P:(b + 1) * P],
                            nat16[:, b * D + k * P: b * D + (k + 1) * P],
                            ident_h,
                        )
                # evacuate to fp8, 3D AP: out [P, KT, ntiles*P]
                dst = dstT[:, :, t0 * P:(t0 + ntiles) * P]
                if (t0 // ntiles) % 2 == 0:
                    nc.scalar.copy(dst, pt)
                else:
                    nc.vector.tensor_copy(dst, pt)

            def q2_ops(t, nat):
                junk = junkp.tile([P, D], F32, tag="q2junk")
                if t % 2 == 0:
                    # ACT: junk = (x*sqrt(.5))^2, accum = +q2 (negated later)
                    nc.scalar.activation(
                        junk, nat, AF.Square, scale=HALF_SQRT,
                        accum_out=q2pos[:, t:t + 1],
                    )
                else:
                    # DVE: junk = (x * -0.5) * x, accum = -q2 directly
                    nc.vector.scalar_tensor_tensor(
                        out=junk, in0=nat, scalar=-0.5, in1=nat,
                        op0=A.mult, op1=A.mult,
                        accum_out=nq2[:, t:t + 1],
                    )

            def no_q(t, nat):
                pass

            with tc.tile_pool(name="psA", bufs=2, space="PSUM") as psA:
                # ---- s1 first (stage B needs all of it) ----
                for g in range(RT // GRP):
                    load_convert_transpose(s1, g * GRP, GRP, s1T8, psA, no_q)

                # negq1row: -q1 via neghalf^T @ square(s1T8); replicated to
                # partitions 0/32/64/96 of negq1rep for packed aug matmuls.
                nc.scalar.square(s1sq, s1T8)
                with tc.tile_pool(name="psP", bufs=2, space="PSUM") as psP:
                    for blk in range(SHARD // 512):
                        pr = psP.tile([1, 512], F32, tag="prep")
                        for k in range(KT):
                            nc.tensor.matmul(
                                pr,
                                lhsT=neghalf,
                                rhs=s1sq[:, k, blk * 512:(blk + 1) * 512],
                                start=(k == 0), stop=(k == KT - 1),
                            )
                        for rep in range(4):
                            nc.vector.tensor_copy(
                                negq1rep[32 * rep:32 * rep + 1, blk * 512:(blk + 1) * 512],
                                pr,
                            )

                # ---- s2 groups + interleaved stage B ----
                with tc.tile_pool(name="psB", bufs=3, space="PSUM") as psB:
                    HALF = 512
                    NH = SHARD // HALF  # 2 halves per jt

                    def stage_b(jt_pair):
                        """Emit PE+ACT+DVE work for jt_pair = [jt, jt+1]."""
                        pgs = []
                        for jt in jt_pair:
                            pg = psB.tile([P, SHARD], F32, tag="gram")
                            pgs.append((jt, pg))
                            for h in range(NH):
                                sl = slice(h * HALF, (h + 1) * HALF)
                                if use_doublerow:
                                    nc.tensor.matmul(
                                        pg[:, sl],
                                        lhsT=s2T8[:, :, jt * P:(jt + 1) * P],
                                        rhs=s1T8[:, :, sl],
                                        start=True, stop=False,
                                        perf_mode=PM.DoubleRow,
                                    )
                                else:
                                    for k in range(KT):
                                        nc.tensor.matmul(
                                            pg[:, sl],
                                            lhsT=s2T8[:, k, jt * P:(jt + 1) * P],
                                            rhs=s1T8[:, k, sl],
                                            start=(k == 0), stop=False,
                                        )
                        # packed augs: 4 (jt, half) slots at partitions 0/32/64/96
                        slot = 0
                        for jt, pg in pgs:
                            for h in range(NH):
                                sl = slice(h * HALF, (h + 1) * HALF)
                                bp = 32 * slot
                                nc.tensor.matmul(
                                    pg[:, sl],
                                    lhsT=onesrep[bp:bp + 1, :],
                                    rhs=negq1rep[bp:bp + 1, sl],
                                    start=False, stop=True,
                                    tile_position=(bp, 0),
                                )
                                slot += 1
                        # ACT evac + bias, then DVE reductions
                        for jt, pg in pgs:
                            v = vpool.tile([P, SHARD], F16, tag="v")
                            nc.scalar.activation(
                                v, pg, AF.Identity, bias=nq2[:, jt:jt + 1]
                            )
                            if use_ttr:
                                # fold ladder: tt max at 2x shrinks the 1x reduce
                                w = junkp.tile([P, SHARD // 2], F16, tag="w")
                                nc.vector.tensor_tensor(
                                    w, v[:, :SHARD // 2], v[:, SHARD // 2:], A.max
                                )
                                w2 = junkp.tile([P, SHARD // 4], F16, tag="w2")
                                nc.vector.tensor_tensor(
                                    w2, w[:, :SHARD // 4], w[:, SHARD // 4:], A.max
                                )
                                nc.vector.tensor_reduce(
                                    cacc[:, jt:jt + 1], w2, axis=AX.X, op=A.max
                                )
                            else:
                                nc.vector.tensor_reduce(
                                    cacc[:, jt:jt + 1], v, axis=AX.X, op=A.max
                                )
                            nc.vector.tensor_tensor(
                                racc[(jt + 1) % 2], v, racc[jt % 2], A.max
                            )

                    for g in range(JT // GRP):
                        load_convert_transpose(s2, g * GRP, GRP, s2T8, psA, q2_ops)
                        # negate the ACT-accumulated (even) q2 columns: [P, 2] strided
                        nc.vector.tensor_scalar_mul(
                            nq2[:, g * GRP:(g + 1) * GRP:2],
                            q2pos[:, g * GRP:(g + 1) * GRP:2],
                            -1.0,
                        )
                        for half_pair in range(GRP // 2):
                            jt = g * GRP + half_pair * 2
                            stage_b([jt, jt + 1])

            # ---- finalize ----
            with tc.tile_pool(name="psC", bufs=2, space="PSUM") as psC:
                rfin = racc[JT % 2]
                for b in range(RT):
                    ptc = psC.tile([P, P], F16, tag="ptc")
                    nc.tensor.transpose(ptc, rfin[:, b * P:(b + 1) * P], ident_h)
                    nc.vector.tensor_reduce(
                        m1[:, b:b + 1], ptc, axis=AX.X, op=A.max
                    )
                # rowmin_d2 = -2*m1; colmin_d2 = -2*cacc
                nc.vector.tensor_scalar_mul(rfix, m1, -2.0)
                nc.vector.tensor_scalar_mul(cfix, cacc, -2.0)
                nc.sync.dma_start(rowmin[:, :], rfix)
                nc.sync.dma_start(colmin[:, :], cfix)

    nc.compile()
    return nc


_CACHE: dict = {}


def _built(key=("ttr", "dr")):
    if key not in _CACHE:
        _CACHE[key] = build_kernel(
            use_ttr="ttr" in key, use_doublerow="dr" in key,
            use_gpcvt="gpcvt" in key,
        )
    return _CACHE[key]


def run_on_cores(set1: np.ndarray, set2: np.ndarray, variant=("ttr", "dr"), **kw):
    """Run the SPMD kernel; returns (rowmin_d2 [8192], colmin_d2 [8192], results)."""
    from concourse.bass_utils import run_bass_kernel_spmd

    nc = _built(variant)
    in_maps = [
        {
            "s1": np.ascontiguousarray(set1[c * SHARD:(c + 1) * SHARD]),
            "s2": np.ascontiguousarray(set2),
        }
        for c in range(NCORES)
    ]
    res = run_bass_kernel_spmd(nc, in_maps, core_ids=list(range(NCORES)), **kw)
    row_parts, col_parts = [], []
    for c in range(NCORES):
        # [P, T] with element (p, t) = row/col index t*P + p
        row_parts.append(res.results[c]["rowmin"].T.reshape(-1))
        col_parts.append(res.results[c]["colmin"].T.reshape(-1))
    rowmin_d2 = np.concatenate(row_parts)            # [8192], by global row
    colmin_d2 = np.min(np.stack(col_parts), axis=0)  # [8192], min over cores
    return rowmin_d2, colmin_d2, res


def kernel(set1, set2) -> np.ndarray:
    set1 = np.asarray(set1, dtype=np.float32)
    set2 = np.asarray(set2, dtype=np.float32)
    rowmin_d2, colmin_d2, _ = run_on_cores(set1, set2)
    t1 = np.sqrt(np.maximum(rowmin_d2, 0.0), dtype=np.float32).mean(dtype=np.float32)
    t2 = np.sqrt(np.maximum(colmin_d2, 0.0), dtype=np.float32).mean(dtype=np.float32)
    return np.array(np.float32(t1) + np.float32(t2), dtype=np.float32)
